# revision 1
# baseline (speedup 1.0000x reference)
"""Trainium2 Bass kernel for nn_Enhance (vq_codebook): dual-branch 1x1conv+BN+ReLU
codebook-attention enhancement block, data-parallel over batch across 8 NeuronCores.

Self-contained: hardcodes shapes B=16, C=512, H=W=64, D=256 and the batch
sharding (2 images per core). BN training-mode batch stats are made exact via a
small cross-core AllReduce of per-channel sums/sum-of-squares.
"""
import sys

for _p in ("/opt/trn_rl_repo",):
    if _p not in sys.path:
        sys.path.append(_p)

import math
import numpy as np
import ml_dtypes

import concourse.bacc as bacc
import concourse.tile as tile
from concourse import mybir
from concourse.bass_utils import run_bass_kernel_spmd
from concourse.masks import make_identity

F16 = mybir.dt.float16
F32 = mybir.dt.float32
AF = mybir.ActivationFunctionType
OP = mybir.AluOpType

N_CORES = 8
B, C, H, W, D = 16, 512, 64, 64, 256
S = H * W                      # 4096 spatial positions per image
B_LOC = B // N_CORES           # 2 images per core
ST = 512                       # spatial tile (free dim)
NT = S // ST                   # 8 spatial tiles per image
NCB = C // 128                 # 4 channel blocks
NDB = D // 128                 # 2 codebook blocks
ISC = 1.0 / math.sqrt(C)
N_TOT = float(B * S)           # BN stat count (full batch)
EPS = 1e-5


def build_bass(use_collective=True, variant='full'):
    """Build the per-core Bass program.

    use_collective=False swaps the stats AllReduce for a local DMA copy
    (single-core timing simulation only — numerically it would compute
    per-core-batch BN stats)."""
    nc = bacc.Bacc(None, target_bir_lowering=False, num_devices=N_CORES)

    # ---- I/O ----------------------------------------------------------------
    # x shards pre-arranged on host as [b, t, cb, p, s] (contiguous 512KB tiles)
    xh = nc.dram_tensor("xh", [B_LOC, NT, NCB, 128, ST], F16, kind="ExternalInput")
    # weights pre-transposed on host: wgt/wct = conv_w.T -> (cb, p, c_out)
    wgt_d = nc.dram_tensor("wgt", [NCB, 128, C], F16, kind="ExternalInput")
    wct_d = nc.dram_tensor("wct", [NCB, 128, C], F16, kind="ExternalInput")
    q_d = nc.dram_tensor("q", [NCB, 128, D], F16, kind="ExternalInput")    # Q (c,d)
    qt_d = nc.dram_tensor("qt", [NDB, 128, C], F16, kind="ExternalInput")  # Q.T (d,c)
    # bn params: columns [gamma_g(4) gamma_c(4) beta_g(4) beta_c(4)] by ob
    bnp_d = nc.dram_tensor("bnp", [128, 16], F32, kind="ExternalInput")
    # one-hot row selector for partition-broadcast: sel[k, ob*128+m] = (k==ob)
    sel_d = nc.dram_tensor("sel", [NCB, NCB * 128], F32, kind="ExternalInput")
    out_d = nc.dram_tensor("out", [B_LOC, NT, NCB, 128, ST], F16, kind="ExternalOutput")

    xh_ap = xh.ap()
    out_ap = out_d.ap()

    with tile.TileContext(nc) as tc:
        with (
            tc.tile_pool(name="const", bufs=1) as constp,
            tc.tile_pool(name="big", bufs=1) as bigp,
            tc.tile_pool(name="small", bufs=1) as smallp,
            tc.tile_pool(name="dram", bufs=1, space="DRAM") as dramp,
        ):
            # ---- constants / weights in SBUF -------------------------------
            wgt = constp.tile([128, NCB, C], F16)
            wct = constp.tile([128, NCB, C], F16)
            qsb = constp.tile([128, NCB, D], F16)
            qtsb = constp.tile([128, NDB, C], F16)
            bnp = constp.tile([128, 16], F32)
            nc.sync.dma_start(out=wgt, in_=wgt_d.ap().rearrange("cb p c -> p cb c"))
            nc.sync.dma_start(out=wct, in_=wct_d.ap().rearrange("cb p c -> p cb c"))
            nc.sync.dma_start(out=qsb, in_=q_d.ap().rearrange("cb p d -> p cb d"))
            nc.sync.dma_start(out=qtsb, in_=qt_d.ap().rearrange("db p c -> p db c"))
            nc.sync.dma_start(out=bnp, in_=bnp_d.ap())
            ones_h = constp.tile([128, 128], F16)
            nc.vector.memset(ones_h, 1.0)
            # row-selector for partition-broadcast of g (host-provided)
            sel_f = constp.tile([NCB, NCB * 128], F32)
            nc.sync.dma_start(out=sel_f, in_=sel_d.ap())
            ident_h = constp.tile([128, 128], F16)
            make_identity(nc, ident_h)
            ident_f = constp.tile([128, 128], F32)
            make_identity(nc, ident_f)

            # ---- persistent activation stores (fp16) ------------------------
            gx_sb = bigp.tile([128, NCB, B_LOC, S], F16, tag="gx")
            cx_sb = bigp.tile([128, NCB, B_LOC, S], F16, tag="cx")

            # stat slots per (branch*4+ob, b*NT+t)
            sum_slots = smallp.tile([128, 2 * NCB, B_LOC * NT], F32, tag="sum_slots")
            ssq_slots = smallp.tile([128, 2 * NCB, B_LOC * NT], F32, tag="ssq_slots")
            stats_sb = smallp.tile([128, 16], F32, tag="stats")  # [ssq(8) | sum(8)]
            tot_sb = smallp.tile([128, 16], F32, tag="tot")

            # =================================================================
            # Pass 1: convs, stores, stat partials
            # =================================================================
            with (
                tc.tile_pool(name="p1x", bufs=4) as p1x,
                tc.tile_pool(name="p1s", bufs=2) as p1s,
                tc.tile_pool(name="cv", bufs=8, space="PSUM") as cvp,
            ):
                for b in range(B_LOC):
                    for t in range(NT):
                        bt = b * NT + t
                        xt = p1x.tile([128, NCB, ST], F16, tag="xt")
                        nc.sync.dma_start(
                            out=xt, in_=xh_ap[b, t].rearrange("cb p s -> p cb s")
                        )
                        # both conv branches, paired output blocks (2 PSUM banks)
                        for br, (wsb, store) in enumerate(
                            ((wgt, gx_sb), (wct, cx_sb))
                        ):
                            for ob in range(NCB):  # single-bank psum per block
                                ps = cvp.tile([128, ST], F32, tag="cv")
                                for cb in range(NCB):
                                    nc.tensor.matmul(
                                        out=ps,
                                        lhsT=wsb[:, cb, ob * 128 : (ob + 1) * 128],
                                        rhs=xt[:, cb, :],
                                        start=(cb == 0),
                                        stop=(cb == NCB - 1),
                                    )
                                # drain with per-channel sum accum (DVE) +
                                # sum-of-squares accum from store (ACT)
                                sidx = br * NCB + ob
                                st_slice = store[:, ob, b, t * ST : (t + 1) * ST]
                                if variant == "nostat":
                                    nc.vector.tensor_copy(
                                        out=st_slice, in_=ps
                                    )
                                    continue
                                nc.vector.tensor_scalar(
                                    out=st_slice,
                                    in0=ps,
                                    scalar1=1.0,
                                    scalar2=0.0,
                                    op0=OP.mult,
                                    op1=OP.add,
                                    accum_out=sum_slots[:, sidx, bt : bt + 1],
                                )
                                sqs = p1s.tile([128, ST], F16, tag="sq_scr")
                                nc.scalar.activation(
                                    out=sqs,
                                    in_=st_slice,
                                    func=AF.Square,
                                    accum_out=ssq_slots[:, sidx, bt : bt + 1],
                                )

            # reduce slots -> stats vector
            nc.vector.tensor_reduce(
                out=stats_sb[:, 0:8], in_=ssq_slots, axis=mybir.AxisListType.X, op=OP.add
            )
            nc.vector.tensor_reduce(
                out=stats_sb[:, 8:16], in_=sum_slots, axis=mybir.AxisListType.X, op=OP.add
            )

            # ---- cross-core AllReduce of stats ------------------------------
            cc_in = dramp.tile([128, 16], F32, tag="cc_in")
            cc_out = dramp.tile([128, 16], F32, tag="cc_out")
            nc.sync.dma_start(out=cc_in, in_=stats_sb)
            if use_collective:
                nc.gpsimd.collective_compute(
                    "AllReduce",
                    OP.add,
                    replica_groups=[list(range(N_CORES))],
                    ins=[cc_in.opt()],
                    outs=[cc_out.opt()],
                )
            else:
                nc.sync.dma_start(out=cc_out, in_=cc_in)
            nc.sync.dma_start(out=tot_sb, in_=cc_out)

            if variant not in ("p1", "nostat"):
                # =================================================================
                # Pass 2: BN coefs, gating branch, codebook attention, residual
                # =================================================================
                with (
                    tc.tile_pool(name="p2", bufs=2) as p2,
                    tc.tile_pool(name="p2d", bufs=5) as p2d,
                    tc.tile_pool(name="p2r", bufs=3) as p2r,
                    tc.tile_pool(name="p2c", bufs=3) as p2c,
                    tc.tile_pool(name="tp", bufs=2, space="PSUM") as tpp,
                    tc.tile_pool(name="sp", bufs=1, space="PSUM") as spp,
                    tc.tile_pool(name="c2", bufs=2, space="PSUM") as c2p,
                    tc.tile_pool(name="gt", bufs=1, space="PSUM") as gtp,
                ):
                    mean = smallp.tile([128, 8], F32, tag="mean")
                    nc.vector.tensor_scalar(
                        out=mean, in0=tot_sb[:, 8:16], scalar1=1.0 / N_TOT, scalar2=None,
                        op0=OP.mult,
                    )
                    ex2 = smallp.tile([128, 8], F32, tag="ex2")
                    nc.vector.tensor_scalar(
                        out=ex2, in0=tot_sb[:, 0:8], scalar1=1.0 / N_TOT, scalar2=None,
                        op0=OP.mult,
                    )
                    var = smallp.tile([128, 8], F32, tag="var")
                    nc.vector.tensor_mul(out=var, in0=mean, in1=mean)
                    nc.vector.tensor_sub(out=var, in0=ex2, in1=var)
                    # rstd = 1/sqrt(var+eps); a = gamma*rstd; b = beta - mean*a
                    sd = smallp.tile([128, 8], F32, tag="sd")
                    eps_t = smallp.tile([128, 1], F32, tag="eps")
                    nc.vector.memset(eps_t, EPS)
                    nc.scalar.activation(out=sd, in_=var, func=AF.Sqrt, bias=eps_t)
                    nc.vector.reciprocal(out=sd, in_=sd)
                    a_sb = smallp.tile([128, 8], F32, tag="a_sb")
                    b_sb = smallp.tile([128, 8], F32, tag="b_sb")
                    nc.vector.tensor_mul(out=a_sb, in0=sd, in1=bnp[:, 0:8])
                    nc.vector.tensor_mul(out=b_sb, in0=mean, in1=a_sb)
                    nc.vector.tensor_sub(out=b_sb, in0=bnp[:, 8:16], in1=b_sb)

                    def _c_branch(b, t, qtg):
                        cxn = p2c.tile([128, NCB, ST], F16, tag="cxn")
                        for ob in range(NCB):
                            if ob < 3:  # DVE path to unload ACT
                                caf = p2.tile([128, ST], F16, tag="caf_scr")
                                nc.vector.tensor_scalar(
                                    out=caf,
                                    in0=cx_sb[:, ob, b, t * ST : (t + 1) * ST],
                                    scalar1=a_sb[:, NCB + ob : NCB + ob + 1],
                                    scalar2=b_sb[:, NCB + ob : NCB + ob + 1],
                                    op0=OP.mult,
                                    op1=OP.add,
                                )
                                nc.vector.tensor_scalar_max(
                                    out=cxn[:, ob, :], in0=caf, scalar1=0.0
                                )
                            else:
                                nc.scalar.activation(
                                    out=cxn[:, ob, :],
                                    in_=cx_sb[:, ob, b, t * ST : (t + 1) * ST],
                                    func=AF.Relu,
                                    scale=a_sb[:, NCB + ob : NCB + ob + 1],
                                    bias=b_sb[:, NCB + ob : NCB + ob + 1],
                                )
                        tps = tpp.tile([128, NDB, ST], F32, tag="tps")
                        for db in range(NDB):
                            for cb in range(NCB):
                                nc.tensor.matmul(
                                    out=tps[:, db, :],
                                    lhsT=qsb[:, cb, db * 128 : (db + 1) * 128],
                                    rhs=cxn[:, cb, :],
                                    start=(cb == 0),
                                    stop=(cb == NCB - 1),
                                )
                        e = p2d.tile([128, NDB, ST], F16, tag="e")
                        nc.scalar.activation(out=e, in_=tps, func=AF.Exp, scale=ISC)
                        sps = spp.tile([128, ST], F32, tag="colsum")
                        for db in range(NDB):
                            nc.tensor.matmul(
                                out=sps,
                                lhsT=ones_h,
                                rhs=e[:, db, :],
                                start=(db == 0),
                                stop=(db == NDB - 1),
                            )
                        rcp = p2r.tile([128, ST], F16, tag="rcp")
                        with nc.allow_low_precision(reason="softmax denom fp16 ok"):
                            nc.vector.reciprocal(out=rcp, in_=sps)
                        ep = p2d.tile([128, NDB, ST], F16, tag="ep")
                        for db in range(NDB):
                            nc.vector.tensor_mul(
                                out=ep[:, db, :], in0=e[:, db, :], in1=rcp
                            )
                        xt2 = p2.tile([128, NCB, ST], F16, tag="xt2")
                        nc.sync.dma_start(
                            out=xt2, in_=xh_ap[b, t].rearrange("cb p s -> p cb s")
                        )
                        out_sb = p2.tile([128, NCB, ST], F16, tag="out_sb")
                        for ob in range(NCB):
                            c2 = c2p.tile([128, ST], F32, tag="c2")
                            for db in range(NDB):
                                nc.tensor.matmul(
                                    out=c2,
                                    lhsT=qtg[:, db, ob * 128 : (ob + 1) * 128],
                                    rhs=ep[:, db, :],
                                    start=(db == 0),
                                    stop=False,
                                )
                            nc.tensor.matmul(
                                out=c2,
                                lhsT=ident_h,
                                rhs=xt2[:, ob, :],
                                start=False,
                                stop=True,
                            )
                            nc.any.tensor_copy(out=out_sb[:, ob, :], in_=c2)
                        nc.sync.dma_start(
                            out=out_ap[b, t].rearrange("cb p s -> p cb s"), in_=out_sb
                        )


                    for b in range(B_LOC):
                        if variant == "nogchain":
                            qtg = p2.tile([128, NDB, C], F16, tag="qtg")
                            for db in range(NDB):
                                nc.vector.tensor_copy(
                                    out=qtg[:, db, :], in_=qtsb[:, db, :]
                                )
                            for t in range(NT):
                                _c_branch(b, t, qtg)
                            continue
                        # ---- gating branch (per image) -------------------------
                        pool_slots = p2.tile([128, NCB, NT], F32, tag="pool_slots")
                        for ob in range(NCB):
                            for t in range(NT):
                                src = gx_sb[:, ob, b, t * ST : (t + 1) * ST]
                                if variant == "gpoolact":
                                    use_act = True
                                elif variant == "gpooldve":
                                    use_act = False
                                else:
                                    use_act = t < 2
                                if use_act:  # ACT path: fused relu(a*x+b) + accum
                                    gxn = p2.tile([128, ST], F16, tag="gxn_scr")
                                    nc.scalar.activation(
                                        out=gxn,
                                        in_=src,
                                        func=AF.Relu,
                                        scale=a_sb[:, ob : ob + 1],
                                        bias=b_sb[:, ob : ob + 1],
                                        accum_out=pool_slots[:, ob, t : t + 1],
                                    )
                                else:  # DVE path: affine then max(.,0)+reduce
                                    gaf = p2.tile([128, ST], F16, tag="gaf_scr")
                                    nc.vector.tensor_scalar(
                                        out=gaf,
                                        in0=src,
                                        scalar1=a_sb[:, ob : ob + 1],
                                        scalar2=b_sb[:, ob : ob + 1],
                                        op0=OP.mult,
                                        op1=OP.add,
                                    )
                                    gxn = p2.tile([128, ST], F16, tag="gxn_scr")
                                    nc.vector.tensor_scalar(
                                        out=gxn,
                                        in0=gaf,
                                        scalar1=0.0,
                                        scalar2=0.0,
                                        op0=OP.max,
                                        op1=OP.add,
                                        accum_out=pool_slots[:, ob, t : t + 1],
                                    )
                        if variant in ("gpool", "gpoolact", "gpooldve"):
                            qtg = p2.tile([128, NDB, C], F16, tag="qtg")
                            for db in range(NDB):
                                nc.vector.tensor_copy(
                                    out=qtg[:, db, :], in_=qtsb[:, db, :]
                                )
                            for t in range(NT):
                                _c_branch(b, t, qtg)
                            continue
                        gbar_f = smallp.tile([128, NCB], F32, tag="gbar_f")
                        nc.vector.tensor_reduce(
                            out=gbar_f, in_=pool_slots, axis=mybir.AxisListType.X, op=OP.add
                        )
                        gbar_h = smallp.tile([128, NCB], F16, tag="gbar_h")
                        nc.vector.tensor_copy(out=gbar_h, in_=gbar_f)
                        tg = gtp.tile([128, 8], F32, tag="gtiny")
                        for db in range(NDB):
                            for cb in range(NCB):
                                nc.tensor.matmul(
                                    out=tg[:, db : db + 1],
                                    lhsT=qsb[:, cb, db * 128 : (db + 1) * 128],
                                    rhs=gbar_h[:, cb : cb + 1],
                                    start=(cb == 0),
                                    stop=(cb == NCB - 1),
                                )
                        eg = smallp.tile([128, NDB], F16, tag="eg")
                        nc.scalar.activation(
                            out=eg, in_=tg[:, 0:NDB], func=AF.Exp, scale=ISC / S
                        )
                        sg = gtp.tile([128, 8], F32, tag="gtiny")
                        for db in range(NDB):
                            nc.tensor.matmul(
                                out=sg[:, 0:1],
                                lhsT=ones_h,
                                rhs=eg[:, db : db + 1],
                                start=(db == 0),
                                stop=(db == NDB - 1),
                            )
                        rcg = smallp.tile([128, 1], F32, tag="rcg")
                        nc.vector.reciprocal(out=rcg, in_=sg[:, 0:1])
                        aff = smallp.tile([128, NDB], F16, tag="aff")
                        nc.vector.tensor_scalar_mul(out=aff, in0=eg, scalar1=rcg)
                        gpre = gtp.tile([128, 8], F32, tag="gtiny")
                        for ob in range(NCB):
                            for db in range(NDB):
                                nc.tensor.matmul(
                                    out=gpre[:, ob : ob + 1],
                                    lhsT=qtsb[:, db, ob * 128 : (ob + 1) * 128],
                                    rhs=aff[:, db : db + 1],
                                    start=(db == 0),
                                    stop=(db == NDB - 1),
                                )
                        g_f = smallp.tile([128, NCB], F32, tag="g_f")
                        nc.scalar.activation(out=g_f, in_=gpre[:, 0:NCB], func=AF.Sigmoid)
                        if variant == "nog":
                            qtg = p2.tile([128, NDB, C], F16, tag="qtg")
                            for db in range(NDB):
                                nc.vector.tensor_copy(out=qtg[:, db, :], in_=qtsb[:, db, :])
                            continue_g = True
                        else:
                            continue_g = False
                        # broadcast g across partitions: transpose + rank-1 matmul
                        gT = spp.tile([NCB, 128], F32, tag="colsum")
                        nc.tensor.transpose(out=gT, in_=g_f, identity=ident_f)
                        gT_sb = smallp.tile([NCB, 128], F32, tag="gT_sb")
                        nc.vector.tensor_copy(out=gT_sb, in_=gT)
                        bc = spp.tile([128, C], F32, tag="colsum")
                        for ob in range(NCB):
                            nc.tensor.matmul(
                                out=bc[:, ob * 128 : (ob + 1) * 128],
                                lhsT=sel_f[:, ob * 128 : (ob + 1) * 128],
                                rhs=gT_sb[0:NCB, :],
                                start=True,
                                stop=True,
                            )
                        if not continue_g:
                            qtg = p2.tile([128, NDB, C], F16, tag="qtg")
                            for db in range(NDB):
                                nc.vector.tensor_mul(
                                    out=qtg[:, db, :], in0=qtsb[:, db, :], in1=bc
                                )

                        # ---- channel branch, spatially tiled -------------------
                        if variant == "gonly":
                            continue
                        for t in range(NT):
                            _c_branch(b, t, qtg)

    nc.finalize()
    return nc


_NC_CACHE = None


def _get_nc():
    global _NC_CACHE
    if _NC_CACHE is None:
        _NC_CACHE = build_bass()
    return _NC_CACHE


def kernel(x, weight_global, conv_g_w, bn_g_gamma, bn_g_beta, conv_c_w,
           bn_c_gamma, bn_c_beta):
    x = np.asarray(x, np.float32)
    weight_global = np.asarray(weight_global, np.float32)
    conv_g_w = np.asarray(conv_g_w, np.float32)
    conv_c_w = np.asarray(conv_c_w, np.float32)
    bn_g_gamma = np.asarray(bn_g_gamma, np.float32)
    bn_g_beta = np.asarray(bn_g_beta, np.float32)
    bn_c_gamma = np.asarray(bn_c_gamma, np.float32)
    bn_c_beta = np.asarray(bn_c_beta, np.float32)

    # QR-orthogonal codebook (host; replicated param per sharding hint)
    Q = np.linalg.qr(weight_global + 1e-8)[0]  # (C, D) fp32

    # host data prep: (B,C,H,W) -> (b, t, cb, p, s)
    xr = x.reshape(B, NCB, 128, NT, ST).transpose(0, 3, 1, 2, 4)
    xh_all = np.ascontiguousarray(xr).astype(np.float16)

    wgt = np.ascontiguousarray(conv_g_w.T).reshape(NCB, 128, C).astype(np.float16)
    wct = np.ascontiguousarray(conv_c_w.T).reshape(NCB, 128, C).astype(np.float16)
    qh = np.ascontiguousarray(Q).reshape(NCB, 128, D).astype(np.float16)
    qth = np.ascontiguousarray(Q.T).reshape(NDB, 128, C).astype(np.float16)
    bnp = np.concatenate(
        [
            bn_g_gamma.reshape(NCB, 128).T,
            bn_c_gamma.reshape(NCB, 128).T,
            bn_g_beta.reshape(NCB, 128).T,
            bn_c_beta.reshape(NCB, 128).T,
        ],
        axis=1,
    ).astype(np.float32)
    bnp = np.ascontiguousarray(bnp)
    sel_np = np.zeros((NCB, NCB * 128), np.float32)
    for ob in range(NCB):
        sel_np[ob, ob * 128 : (ob + 1) * 128] = 1.0

    nc = _get_nc()
    in_maps = []
    for c in range(N_CORES):
        in_maps.append(
            {
                "xh": np.ascontiguousarray(xh_all[c * B_LOC : (c + 1) * B_LOC]),
                "wgt": wgt,
                "wct": wct,
                "q": qh,
                "qt": qth,
                "bnp": bnp,
                "sel": sel_np,
            }
        )
    res = run_bass_kernel_spmd(nc, in_maps, core_ids=list(range(N_CORES)))

    # gather: (b, t, cb, p, s) -> (B, C, H, W)
    parts = [res.results[c]["out"] for c in range(N_CORES)]
    o = np.concatenate(parts, axis=0).astype(np.float32)  # (B, NT, NCB, 128, ST)
    o = o.transpose(0, 2, 3, 1, 4).reshape(B, C, H, W)
    return np.ascontiguousarray(o)

